# revision 2
# baseline (speedup 1.0000x reference)
"""Trainium2 Bass kernel for nn_BioConvolution (locally-connected conv,
stride == kernel, unshared per-location filters).

  X [64, 64, 64, 64] f32 (N, H, W, Cin), filters [1, 256, 4, 4, 64, 128],
  bias [128]  ->  out [64, 16, 16, 128] f32
  out[n, r, c, f] = relu(sum_{i,j,ch} X[n, 4r+i, 4c+j, ch]
                         * filters[0, r*16+c, i, j, ch, f] + bias[f])

Sharding: the L = 256 location axis is split over 8 NeuronCores (the
natural spatial/tensor split — weights are unshared per location, so there
is no cross-device reduction).  Core a owns patch rows {2a, 2a+1} = 32
locations, i.e. image rows [8a, 8a+8) of X and filters[0, 32a:32a+32].

Per-location GEMM: patches [64n x 1024K] @ filters [1024K x 128F].  The
kernel is HBM-bandwidth-bound, so dtypes are chosen to minimize traffic
within the 2e-2 rel-err budget: patches in fp16 (~3e-4 err) and filters in
fp8 e3m4 (4 mantissa bits; measured 1.45e-2 scale-relative absmax err,
vs 2.8e-2 for e4m3).  Filters are pre-scaled by 256 on host so their
~N(0, 0.01) values sit in e3m4's normal range [0.25, 15.5]; the 1/256
dequant is folded into the output activation's scale.  Traffic/core:
4.19 MB patches (f16) + 4.19 MB filters (fp8) + 0.52 MB output (f16).

On-device dataflow per core, pipelined in groups of 2 columns:
  1. HW DMA-transpose (xbar) loads the patch block [128 batch-rows x 2048]
     directly transposed into SBUF as patchesT tiles [128 K-rows, batch]
     (the tensor engine contracts over the partition dim; the 2-byte xbar
     transpose does this at DMA time) on the sync-queue ring.
  2. fp8 filters stream in [q, (c, r, kk, f)] layout on the scalar ring.
  3. Per location: 8 accumulating matmuls with the filter tile stationary
     (lhsT [128K, 128F]) and the patch tile moving (rhs [128K, 64n]) into
     PSUM [128F, 64n] — 64 moving rows per matmul instead of 128, halving
     tensor-engine time vs the patches-stationary orientation.
  4. ReLU on ScalarE (PSUM -> SBUF) with per-partition f32 bias (partition
     dim is now F) and scale=1/256 applying bias + fp8 dequant for free;
     per-group output DMA on the SWDGE ring (f16; upcast to f32 on host).
No collectives are needed; the host concatenates the 8 location shards.
"""
import numpy as np
import ml_dtypes

N, H, W, C = 64, 64, 64, 64
FH, FW, F = 4, 4, 128
R = Cc = 16          # 16x16 patch grid
K = FH * FW * C      # 1024 contraction
KK = K // 128        # 8 k-tiles of 128
NC_CORES = 8
RPC = R // NC_CORES  # patch rows per core = 2
W_SCALE = 256.0      # filters pre-scale into e3m4 normal range

_compiled = {}


def _host_shards(X, filters, bias, dtype):
    """Per-core input maps. Host work is sharding + layout: slice rows,
    regroup (row-pair, batch) onto SBUF partitions, cast to f16/fp8."""
    X = np.asarray(X, np.float32)
    filters = np.asarray(filters, np.float32)
    bias = np.asarray(bias, np.float32)
    f8 = ml_dtypes.float8_e3m4

    # B[r, n, c, K]: patch row r, batch n, column c, K = (i*4+j)*64+ch
    A = X.reshape(N, R, FH, Cc, FW, C)                     # n r i c j ch
    B = np.ascontiguousarray(A.transpose(1, 0, 3, 2, 4, 5)).reshape(R, N, Cc, K)
    # filters q-major per core: fl[q, c, r_local, kk, f], K = kk*128+q
    flt = filters[0].reshape(8, RPC, Cc, KK, 128, F)       # a r c kk q f
    fl9 = flt.transpose(0, 4, 2, 1, 3, 5)                  # a q c r kk f
    fl9 = np.clip(fl9 * W_SCALE, -15.5, 15.5).astype(f8)

    in_maps = []
    for a in range(NC_CORES):
        xs = B[2 * a : 2 * a + 2].reshape(128, Cc, K).astype(dtype)
        fl = np.ascontiguousarray(fl9[a]).reshape(128, Cc, RPC * KK * F)
        in_maps.append({
            "xs": np.ascontiguousarray(xs),
            "fl": fl,
            "bias": bias.reshape(F, 1),
        })
    return in_maps


def _build(n_iters=1):
    import concourse.mybir as mybir
    import concourse.tile as tile
    from concourse import bacc

    dtype = mybir.dt.float16
    f8 = mybir.dt.float8e3
    gcols = 2
    nc = bacc.Bacc("TRN2", target_bir_lowering=False, debug=False,
                   num_devices=NC_CORES)
    xs_d = nc.dram_tensor("xs", [128, Cc, K], dtype, kind="ExternalInput").ap()
    fl_d = nc.dram_tensor("fl", [128, Cc, RPC * KK * F], f8,
                          kind="ExternalInput").ap()
    bias_d = nc.dram_tensor("bias", [F, 1], mybir.dt.float32,
                            kind="ExternalInput").ap()
    out_d = nc.dram_tensor("out", [F, Cc * RPC * N], dtype,
                           kind="ExternalOutput").ap()
    relu = mybir.ActivationFunctionType.Relu

    with tile.TileContext(nc) as tc:
        with (
            tc.tile_pool(name="const", bufs=1) as const_pool,
            tc.tile_pool(name="pt", bufs=3) as pt_pool,
            tc.tile_pool(name="fl", bufs=3) as fl_pool,
            tc.tile_pool(name="ps", bufs=8, space="PSUM") as ps_pool,
            tc.tile_pool(name="og", bufs=3) as og_pool,
        ):
            bias_t = const_pool.tile([F, 1], mybir.dt.float32, tag="bias")
            nc.scalar.dma_start(bias_t[:], bias_d[:])

            for _ in range(n_iters):
                for c0 in range(0, Cc, gcols):
                    # patch block: one xbar-transposed DMA -> [q, (col kk), p]
                    # with p = r*64 + n
                    pt_sb = pt_pool.tile([128, gcols * KK * 128], dtype,
                                         tag="pt")
                    nc.sync.dma_start(
                        pt_sb[:].rearrange("q (ck p) -> q ck p", p=128),
                        xs_d[:, c0 : c0 + gcols, :],
                        transpose=True,
                    )
                    # fp8 filters: [q, (col, r, kk, f)]
                    fl_sb = fl_pool.tile([128, gcols * RPC * KK * F], f8,
                                         tag="fl")
                    nc.scalar.dma_start(fl_sb[:], fl_d[:, c0 : c0 + gcols])
                    og = og_pool.tile([F, gcols * RPC * N], dtype, tag="og")
                    for ci in range(gcols):
                        for r in range(RPC):
                            ps = ps_pool.tile([F, N], mybir.dt.float32,
                                              tag="ps")
                            for k in range(KK):
                                nc.tensor.matmul(
                                    ps[:],
                                    lhsT=fl_sb[:, ((ci * RPC + r) * KK + k) * F
                                               : ((ci * RPC + r) * KK + k + 1) * F],
                                    rhs=pt_sb[:, (ci * KK + k) * 128 + r * N
                                              : (ci * KK + k) * 128 + r * N + N],
                                    start=(k == 0), stop=(k == KK - 1),
                                )
                            nc.scalar.activation(
                                og[:, (ci * RPC + r) * N : (ci * RPC + r + 1) * N],
                                ps[:], relu, bias=bias_t[:], scale=1.0 / W_SCALE)
                    nc.gpsimd.dma_start(
                        out_d[:, c0 * RPC * N : (c0 + gcols) * RPC * N], og[:])
    nc.compile()
    return nc


def kernel(X, filters, bias):
    from concourse.bass_utils import run_bass_kernel_spmd

    assert X.shape == (N, H, W, C), X.shape
    assert filters.shape == (1, R * Cc, FH, FW, C, F), filters.shape
    assert bias.shape == (F,), bias.shape

    in_maps = _host_shards(X, filters, bias, np.float16)
    if "nc" not in _compiled:
        _compiled["nc"] = _build(n_iters=1)
    res = run_bass_kernel_spmd(_compiled["nc"], in_maps, list(range(NC_CORES)))

    # res[a]["out"]: [F, Cc, RPC, N] -> [N, RPC, Cc, F] per core
    shards = [np.asarray(res.results[a]["out"], np.float32)
              .reshape(F, Cc, RPC, N).transpose(3, 2, 1, 0)
              for a in range(NC_CORES)]
    out = np.stack(shards, axis=1)             # [N, 8, RPC, Cc, F]
    return np.ascontiguousarray(out.reshape(N, R, Cc, F)).astype(np.float32)
